# revision 13
# baseline (speedup 1.0000x reference)
"""GPTQ int4 dequant + matmul + bias + residual for Trainium2, 8 NeuronCores.

Problem (hardcoded): input [4,2048,4096] f32, qweight int32 [512,4096] (8 int4
along K per int32), scales [32,4096], qzeros int32 [32,512] (8 int4 along N),
g_idx = arange(4096)//128 (contiguous groups), bias [4096], residual
[4,2048,4096].  out = x @ dequant(W) + bias + residual.

Sharding: data-parallel over tokens (M = B*S = 8192 rows -> 1024 rows/core);
every core keeps the full weight.  This keeps the x-transpose work (PE
transposes) and input DMA low; the per-core dequant of the full W overlaps
under the fp32r matmuls.

Layout trick: the contraction is processed in a PERMUTED k-order so the packed
int32 weight rows never need replication across partitions.  K splits into 4
super-tiles of 1024 (= 128 packed rows).  Within a super-tile, sub-matmul j
(j = 0..7) contracts k = 1024*T + 8*kp + j over partitions kp = 0..127:
  - rhs_j = ((wq_rows_T >> 4j) & 0xF) * scale  -- wq rows load 1:1 onto
    partitions (plain 2D DMA), one DVE shift/and + one DVE multiply per j
  - lhsT_j = xT slice; built in the prologue by PE-transposing x column slices
    x[:, j::8] so the permuted order falls out of the transpose for free
  - scale rows (group = 8T + kp//16) broadcast across partitions via one
    K=8 indicator matmul (E16^T @ scale_rows) into PSUM per (chunk, T)
  - GPTQ zero-points and bias fold into one rank-33 matmul: out -=
    Xg @ ((qz+1)*s); Xg (per-group sums of x) comes from DVE segmented
    reduces in the prologue, transposed on the PE
"""

import numpy as np

import concourse.bass as bass
import concourse.mybir as mybir
import concourse.tile as tile
from concourse import bacc
from concourse.alu_op_type import AluOpType
from concourse.bass_utils import run_bass_kernel_spmd
from concourse.masks import make_identity

F32 = mybir.dt.float32
F32R = mybir.dt.float16  # matmul dtype: fp16 streams 1cy/col w/ FWL; same 10-bit mantissa class as fp32r
I32 = mybir.dt.int32

B, S, K, N = 4, 2048, 4096, 4096
PACK = 8
GROUP = 128
G = K // GROUP          # 32 groups
NCORES = 8
M = (B * S) // NCORES   # 1024 rows per core
CHUNK = 512


def _build(M=M, K=K, N=N):
    G = K // GROUP
    MT = M // 128
    TS = K // 1024          # super-tiles of 1024 k
    NC_CH = N // CHUNK
    nc = bacc.Bacc(name="gptq_mm", dynamic_dma_scratch_size=2048)
    xp_d = nc.declare_dram_parameter("xp", [M, K], F32R, isOutput=False)
    xgh_d = nc.declare_dram_parameter("xgh", [G + 1, M], F32, isOutput=False)
    wq_d = nc.declare_dram_parameter("wq", [K // PACK, N], I32, isOutput=False)
    sc_d = nc.declare_dram_parameter("scales", [G, N], F32, isOutput=False)
    nzs_d = nc.declare_dram_parameter("nzs", [G + 1, N], F32, isOutput=False)
    e16_d = nc.declare_dram_parameter("e16", [8, 128], F32, isOutput=False)
    res_d = nc.declare_dram_parameter("resid", [M, N], F32, isOutput=False)
    out_d = nc.declare_dram_parameter("out", [M, N], F32, isOutput=True)

    with tile.TileContext(nc) as tc:
        with tc.tile_pool(name="const", bufs=1) as const:
            xt_all = const.tile([128, TS, 8, M], F32R, tag="xt")   # 128KB/part
            xgt = const.tile([G + 1, M], F32R, tag="xgt")
            e16_sb = const.tile([8, 128], F32R, tag="e16")

            # ------- prologue: DMA-transpose the host-permuted fp16 x -------
            with tc.tile_pool(name="prol", bufs=2) as prol:
                e16_st = prol.tile([8, 128], F32, tag="e16st", bufs=1)
                nc.sync.dma_start(out=e16_st[:], in_=e16_d[:, :])
                nc.scalar.copy(e16_sb[:], e16_st[:])
                xgh_st = prol.tile([G + 1, M], F32, tag="xghst", bufs=1)
                nc.sync.dma_start(out=xgh_st[:], in_=xgh_d[:, :])
                nc.vector.tensor_copy(xgt[:], xgh_st[:])
                for mi in range(MT):
                    ms = slice(mi * 128, (mi + 1) * 128)
                    for t in range(TS):
                        for j in range(8):
                            base = (t * 8 + j) * 128
                            nc.sync.dma_start(
                                out=xt_all[:, t, j, ms],
                                in_=xp_d[ms, base:base + 128],
                                transpose=True,
                            )

            # ---------------- steady state: chunks of CHUNK cols ----------------
            with (
                tc.tile_pool(name="wdq", bufs=36) as wdqp,
                tc.tile_pool(name="pk", bufs=3) as pkp,
                tc.tile_pool(name="u", bufs=3) as up,
                tc.tile_pool(name="stg", bufs=1) as stgp,
                tc.tile_pool(name="eout", bufs=2) as eoutp,
                tc.tile_pool(name="psums", bufs=2, space="PSUM") as psums,
                tc.tile_pool(name="psumm", bufs=4, space="PSUM") as psumm,
            ):
                for c in range(NC_CH):
                    cs = slice(c * CHUNK, (c + 1) * CHUNK)
                    # stage nzs for this chunk, cast to f32r
                    nzs_st = stgp.tile([G + 1, CHUNK], F32, tag="nzst")
                    nc.sync.dma_start(out=nzs_st[:], in_=nzs_d[:, cs])
                    nzs_r = stgp.tile([G + 1, CHUNK], F32R, tag="nzsr")
                    nc.scalar.copy(nzs_r[:], nzs_st[:])

                    wdq_tiles = []
                    for t in range(TS):
                        pk = pkp.tile([128, CHUNK], I32, tag="pk")
                        nc.sync.dma_start(
                            out=pk[:], in_=wq_d[128 * t:128 * (t + 1), cs]
                        )
                        # stage the 8 scale rows of this super-tile, cast to f32r
                        s8 = stgp.tile([8, CHUNK], F32, tag="s8", bufs=2)
                        nc.sync.dma_start(out=s8[:], in_=sc_d[8 * t:8 * t + 8, cs])
                        s8r = stgp.tile([8, CHUNK], F32R, tag="s8r", bufs=2)
                        nc.scalar.copy(s8r[:], s8[:])
                        # scale broadcast: ps_s[p, n] = scales[8t + p//16, n]
                        ps_s = psums.tile([128, CHUNK], F32, tag="pss")
                        nc.tensor.matmul(
                            ps_s[:], lhsT=e16_sb[:], rhs=s8r[:],
                            start=True, stop=True,
                        )
                        for j in range(8):
                            u = up.tile([128, CHUNK], I32, tag="u")
                            nc.vector.tensor_scalar(
                                out=u[:], in0=pk[:],
                                scalar1=4 * j, scalar2=0xF,
                                op0=AluOpType.logical_shift_right,
                                op1=AluOpType.bitwise_and,
                            )
                            wdq = wdqp.tile([128, CHUNK], F32R, tag="wdq")
                            nc.vector.tensor_tensor(
                                out=wdq[:], in0=u[:], in1=ps_s[:], op=AluOpType.mult,
                            )
                            wdq_tiles.append(wdq)

                    for mi in range(MT):
                        ms = slice(mi * 128, (mi + 1) * 128)
                        ps = psumm.tile([128, CHUNK], F32, tag="ps")
                        for t in range(TS):
                            for j in range(8):
                                nc.tensor.matmul(
                                    ps[:],
                                    lhsT=xt_all[:, t, j, ms],
                                    rhs=wdq_tiles[t * 8 + j][:],
                                    start=(t == 0 and j == 0), stop=False,
                                )
                        nc.tensor.matmul(
                            ps[:], lhsT=xgt[:, ms], rhs=nzs_r[:],
                            start=False, stop=True,
                        )
                        # epilogue at 256 granularity to keep tiles small
                        for h in range(CHUNK // 256):
                            hs = slice(h * 256, (h + 1) * 256)
                            hcs = slice(c * CHUNK + h * 256, c * CHUNK + (h + 1) * 256)
                            rt = eoutp.tile([128, 256], F32, tag="rt")
                            nc.sync.dma_start(out=rt[:], in_=res_d[ms, hcs])
                            ob = eoutp.tile([128, 256], F32, tag="ob")
                            nc.vector.tensor_tensor(
                                out=ob[:], in0=ps[:, hs], in1=rt[:], op=AluOpType.add,
                            )
                            nc.sync.dma_start(out=out_d[ms, hcs], in_=ob[:])

    nc.finalize()
    return nc


_NC_CACHE = None


def _get_nc():
    global _NC_CACHE
    if _NC_CACHE is None:
        _NC_CACHE = _build()
    return _NC_CACHE


def _host_prep(weight_scales, weight_zeros, bias):
    G_, N_ = weight_scales.shape
    jj = (np.arange(PACK, dtype=np.int32) * 4)
    qz = ((weight_zeros[:, :, None] >> jj[None, None, :]) & 0xF).reshape(G_, N_)
    nzs = np.concatenate(
        [-(qz + 1).astype(np.float32) * weight_scales, bias[None, :]], axis=0
    ).astype(np.float32)                                     # [G+1, N]
    # e16[r, p] = 1 if p//16 == r else 0
    e16 = np.repeat(np.eye(8, dtype=np.float32), 16, axis=1)  # [8, 128]
    return nzs, e16


def kernel(input, weight, weight_scales, weight_zeros, g_idx, bias, residual):
    input = np.asarray(input, dtype=np.float32)
    weight = np.ascontiguousarray(np.asarray(weight, dtype=np.int32))
    weight_scales = np.ascontiguousarray(np.asarray(weight_scales, dtype=np.float32))
    weight_zeros = np.asarray(weight_zeros, dtype=np.int32)
    g_idx = np.asarray(g_idx, dtype=np.int32)
    bias = np.asarray(bias, dtype=np.float32)
    residual = np.asarray(residual, dtype=np.float32)

    assert input.shape == (B, S, K) and weight.shape == (K // PACK, N)
    assert np.array_equal(g_idx, np.arange(K, dtype=np.int32) // GROUP), \
        "kernel assumes contiguous GPTQ groups (g_idx == arange(K)//group_size)"

    x = input.reshape(B * S, K)
    # permuted fp16 copy: xp[m, t*1024 + j*128 + kp] = x[m, 1024t + 8kp + j]
    xp = np.ascontiguousarray(
        x.reshape(B * S, K // 1024, 128, PACK).swapaxes(2, 3)
        .reshape(B * S, K).astype(np.float16))
    # per-group sums of x (fp32) + ones row, transposed: [G+1, M] per core
    xg_full = x.reshape(B * S, G, GROUP).sum(axis=2, dtype=np.float64).astype(np.float32)
    resid = np.ascontiguousarray(residual.reshape(B * S, N))
    nzs, e16 = _host_prep(weight_scales, weight_zeros, bias)

    nc = _get_nc()
    in_maps = []
    for ci in range(NCORES):
        rs = slice(ci * M, (ci + 1) * M)
        xgh = np.concatenate(
            [xg_full[rs].T, np.ones((1, M), np.float32)], axis=0)
        in_maps.append(dict(
            xp=xp[rs],
            xgh=np.ascontiguousarray(xgh),
            wq=weight,
            scales=weight_scales,
            nzs=nzs,
            e16=e16,
            resid=np.ascontiguousarray(resid[rs]),
        ))

    res = run_bass_kernel_spmd(nc, in_maps, core_ids=list(range(NCORES)))
    out = np.concatenate([r["out"] for r in res.results], axis=0)
    return out.reshape(B, S, N)


# revision 14
# speedup vs baseline: 1.3945x; 1.3945x over previous
"""GPTQ int4 dequant + matmul + bias + residual for Trainium2, 8 NeuronCores.

Problem (hardcoded): input [4,2048,4096] f32, qweight int32 [512,4096] (8 int4
along K per int32), scales [32,4096], qzeros int32 [32,512] (8 int4 along N),
g_idx = arange(4096)//128 (contiguous groups), bias [4096], residual
[4,2048,4096].  out = x @ dequant(W) + bias + residual.

Sharding: data-parallel over tokens (M = B*S = 8192 rows -> 1024 rows/core);
every core keeps the full weight.  This keeps the x-transpose work (PE
transposes) and input DMA low; the per-core dequant of the full W overlaps
under the fp32r matmuls.

Layout trick: the contraction is processed in a PERMUTED k-order so the packed
int32 weight rows never need replication across partitions.  K splits into 4
super-tiles of 1024 (= 128 packed rows).  Within a super-tile, sub-matmul j
(j = 0..7) contracts k = 1024*T + 8*kp + j over partitions kp = 0..127:
  - rhs_j = ((wq_rows_T >> 4j) & 0xF) * scale  -- wq rows load 1:1 onto
    partitions (plain 2D DMA), one DVE shift/and + one DVE multiply per j
  - lhsT_j = xT slice; built in the prologue by PE-transposing x column slices
    x[:, j::8] so the permuted order falls out of the transpose for free
  - scale rows (group = 8T + kp//16) broadcast across partitions via one
    K=8 indicator matmul (E16^T @ scale_rows) into PSUM per (chunk, T)
  - GPTQ zero-points and bias fold into one rank-33 matmul: out -=
    Xg @ ((qz+1)*s); Xg (per-group sums of x) comes from DVE segmented
    reduces in the prologue, transposed on the PE
"""

import numpy as np

import concourse.bass as bass
import concourse.mybir as mybir
import concourse.tile as tile
from concourse import bacc
from concourse.alu_op_type import AluOpType
from concourse.bass_utils import run_bass_kernel_spmd
from concourse.masks import make_identity

F32 = mybir.dt.float32
F32R = mybir.dt.float16  # matmul dtype: fp16 streams 1cy/col w/ FWL; same 10-bit mantissa class as fp32r
I32 = mybir.dt.int32

B, S, K, N = 4, 2048, 4096, 4096
PACK = 8
GROUP = 128
G = K // GROUP          # 32 groups
NCORES = 8
M = (B * S) // NCORES   # 1024 rows per core
CHUNK = 512


def _build(M=M, K=K, N=N):
    G = K // GROUP
    MT = M // 128
    TS = K // 1024          # super-tiles of 1024 k
    NC_CH = N // CHUNK
    nc = bacc.Bacc(name="gptq_mm", dynamic_dma_scratch_size=2048)
    x_d = nc.declare_dram_parameter("x", [M, K], F32, isOutput=False)
    wq_d = nc.declare_dram_parameter("wq", [K // PACK, N], I32, isOutput=False)
    sc_d = nc.declare_dram_parameter("scales", [G, N], F32, isOutput=False)
    nzs_d = nc.declare_dram_parameter("nzs", [G + 1, N], F32, isOutput=False)
    e16_d = nc.declare_dram_parameter("e16", [8, 128], F32, isOutput=False)
    res_d = nc.declare_dram_parameter("resid", [M, N], F32, isOutput=False)
    out_d = nc.declare_dram_parameter("out", [M, N], F32, isOutput=True)

    with tile.TileContext(nc) as tc:
        with tc.tile_pool(name="const", bufs=1) as const:
            xt_all = const.tile([128, TS, 8, M], F32R, tag="xt")   # 128KB/part
            xgt = const.tile([G + 1, M], F32R, tag="xgt")
            e16_sb = const.tile([8, 128], F32R, tag="e16")

            # ---------------- prologue: transpose x, group sums ----------------
            with (
                tc.tile_pool(name="prol", bufs=3) as prol,
                tc.tile_pool(name="prolp", bufs=4, space="PSUM") as prolp,
                tc.tile_pool(name="prolp2", bufs=2, space="PSUM") as prolp2,
            ):
                ident = prol.tile([128, 128], F32, tag="ident", bufs=1)
                make_identity(nc, ident[:])
                e16_st = prol.tile([8, 128], F32, tag="e16st", bufs=1)
                nc.sync.dma_start(out=e16_st[:], in_=e16_d[:, :])
                nc.scalar.copy(e16_sb[:], e16_st[:])
                onesrow_f = prol.tile([1, M], F32, tag="onesrow", bufs=1)
                nc.vector.memset(onesrow_f[:], 1.0)
                nc.vector.tensor_copy(xgt[G:G + 1, :], onesrow_f[:])

                for mi in range(MT):
                    ms = slice(mi * 128, (mi + 1) * 128)
                    xg_sb = prol.tile([128, G], F32, tag="xgsb", bufs=2)
                    for t in range(TS):
                        xl = prol.tile([128, 1024], F32, tag="xl")
                        nc.sync.dma_start(
                            out=xl[:], in_=x_d[ms, t * 1024:(t + 1) * 1024]
                        )
                        xl3 = xl[:].rearrange("p (kp j) -> p kp j", j=8)
                        for j in range(8):
                            pt = prolp.tile([128, 128], F32, tag="pt")
                            nc.tensor.transpose(pt[:], xl3[:, :, j], ident[:])
                            dst = xt_all[:, t, j, ms]
                            if j % 2 == 0:
                                nc.vector.tensor_copy(dst, pt[:])
                            else:
                                nc.scalar.copy(dst, pt[:])
                        # per-group sums (groups 8t .. 8t+8)
                        nc.vector.tensor_reduce(
                            out=xg_sb[:, 8 * t:8 * t + 8],
                            in_=xl[:].rearrange("p (s v) -> p s v", v=GROUP),
                            axis=mybir.AxisListType.X,
                            op=AluOpType.add,
                        )
                    pxt = prolp2.tile([G, 128], F32, tag="pxt")
                    nc.tensor.transpose(pxt[:], xg_sb[:], ident[:])
                    nc.vector.tensor_copy(xgt[0:G, ms], pxt[:])

            # ---------------- steady state: chunks of CHUNK cols ----------------
            with (
                tc.tile_pool(name="wdq", bufs=36) as wdqp,
                tc.tile_pool(name="pk", bufs=3) as pkp,
                tc.tile_pool(name="u", bufs=3) as up,
                tc.tile_pool(name="stg", bufs=1) as stgp,
                tc.tile_pool(name="eout", bufs=2) as eoutp,
                tc.tile_pool(name="psums", bufs=2, space="PSUM") as psums,
                tc.tile_pool(name="psumm", bufs=4, space="PSUM") as psumm,
            ):
                for c in range(NC_CH):
                    cs = slice(c * CHUNK, (c + 1) * CHUNK)
                    # stage nzs for this chunk, cast to f32r
                    nzs_st = stgp.tile([G + 1, CHUNK], F32, tag="nzst")
                    nc.sync.dma_start(out=nzs_st[:], in_=nzs_d[:, cs])
                    nzs_r = stgp.tile([G + 1, CHUNK], F32R, tag="nzsr")
                    nc.scalar.copy(nzs_r[:], nzs_st[:])

                    wdq_tiles = []
                    for t in range(TS):
                        pk = pkp.tile([128, CHUNK], I32, tag="pk")
                        nc.sync.dma_start(
                            out=pk[:], in_=wq_d[128 * t:128 * (t + 1), cs]
                        )
                        # stage the 8 scale rows of this super-tile, cast to f32r
                        s8 = stgp.tile([8, CHUNK], F32, tag="s8", bufs=2)
                        nc.sync.dma_start(out=s8[:], in_=sc_d[8 * t:8 * t + 8, cs])
                        s8r = stgp.tile([8, CHUNK], F32R, tag="s8r", bufs=2)
                        nc.scalar.copy(s8r[:], s8[:])
                        # scale broadcast: ps_s[p, n] = scales[8t + p//16, n]
                        ps_s = psums.tile([128, CHUNK], F32, tag="pss")
                        nc.tensor.matmul(
                            ps_s[:], lhsT=e16_sb[:], rhs=s8r[:],
                            start=True, stop=True,
                        )
                        for j in range(8):
                            u = up.tile([128, CHUNK], I32, tag="u")
                            nc.vector.tensor_scalar(
                                out=u[:], in0=pk[:],
                                scalar1=4 * j, scalar2=0xF,
                                op0=AluOpType.logical_shift_right,
                                op1=AluOpType.bitwise_and,
                            )
                            wdq = wdqp.tile([128, CHUNK], F32R, tag="wdq")
                            nc.vector.tensor_tensor(
                                out=wdq[:], in0=u[:], in1=ps_s[:], op=AluOpType.mult,
                            )
                            wdq_tiles.append(wdq)

                    for mi in range(MT):
                        ms = slice(mi * 128, (mi + 1) * 128)
                        ps = psumm.tile([128, CHUNK], F32, tag="ps")
                        for t in range(TS):
                            for j in range(8):
                                nc.tensor.matmul(
                                    ps[:],
                                    lhsT=xt_all[:, t, j, ms],
                                    rhs=wdq_tiles[t * 8 + j][:],
                                    start=(t == 0 and j == 0), stop=False,
                                )
                        nc.tensor.matmul(
                            ps[:], lhsT=xgt[:, ms], rhs=nzs_r[:],
                            start=False, stop=True,
                        )
                        # epilogue at 256 granularity to keep tiles small
                        for h in range(CHUNK // 256):
                            hs = slice(h * 256, (h + 1) * 256)
                            hcs = slice(c * CHUNK + h * 256, c * CHUNK + (h + 1) * 256)
                            rt = eoutp.tile([128, 256], F32, tag="rt")
                            nc.sync.dma_start(out=rt[:], in_=res_d[ms, hcs])
                            ob = eoutp.tile([128, 256], F32, tag="ob")
                            nc.vector.tensor_tensor(
                                out=ob[:], in0=ps[:, hs], in1=rt[:], op=AluOpType.add,
                            )
                            nc.sync.dma_start(out=out_d[ms, hcs], in_=ob[:])

    nc.finalize()
    return nc


_NC_CACHE = None


def _get_nc():
    global _NC_CACHE
    if _NC_CACHE is None:
        _NC_CACHE = _build()
    return _NC_CACHE


def _host_prep(weight_scales, weight_zeros, bias):
    G_, N_ = weight_scales.shape
    jj = (np.arange(PACK, dtype=np.int32) * 4)
    qz = ((weight_zeros[:, :, None] >> jj[None, None, :]) & 0xF).reshape(G_, N_)
    nzs = np.concatenate(
        [-(qz + 1).astype(np.float32) * weight_scales, bias[None, :]], axis=0
    ).astype(np.float32)                                     # [G+1, N]
    # e16[r, p] = 1 if p//16 == r else 0
    e16 = np.repeat(np.eye(8, dtype=np.float32), 16, axis=1)  # [8, 128]
    return nzs, e16


def kernel(input, weight, weight_scales, weight_zeros, g_idx, bias, residual):
    input = np.asarray(input, dtype=np.float32)
    weight = np.ascontiguousarray(np.asarray(weight, dtype=np.int32))
    weight_scales = np.ascontiguousarray(np.asarray(weight_scales, dtype=np.float32))
    weight_zeros = np.asarray(weight_zeros, dtype=np.int32)
    g_idx = np.asarray(g_idx, dtype=np.int32)
    bias = np.asarray(bias, dtype=np.float32)
    residual = np.asarray(residual, dtype=np.float32)

    assert input.shape == (B, S, K) and weight.shape == (K // PACK, N)
    assert np.array_equal(g_idx, np.arange(K, dtype=np.int32) // GROUP), \
        "kernel assumes contiguous GPTQ groups (g_idx == arange(K)//group_size)"

    x = np.ascontiguousarray(input.reshape(B * S, K))
    resid = np.ascontiguousarray(residual.reshape(B * S, N))
    nzs, e16 = _host_prep(weight_scales, weight_zeros, bias)

    nc = _get_nc()
    in_maps = []
    for ci in range(NCORES):
        rs = slice(ci * M, (ci + 1) * M)
        in_maps.append(dict(
            x=np.ascontiguousarray(x[rs]),
            wq=weight,
            scales=weight_scales,
            nzs=nzs,
            e16=e16,
            resid=np.ascontiguousarray(resid[rs]),
        ))

    res = run_bass_kernel_spmd(nc, in_maps, core_ids=list(range(NCORES)))
    out = np.concatenate([r["out"] for r in res.results], axis=0)
    return out.reshape(B, S, N)


# revision 15
# speedup vs baseline: 1.4057x; 1.0080x over previous
"""GPTQ int4 dequant + matmul + bias + residual for Trainium2, 8 NeuronCores.

Problem (hardcoded): input [4,2048,4096] f32, qweight int32 [512,4096] (8 int4
along K per int32), scales [32,4096], qzeros int32 [32,512] (8 int4 along N),
g_idx = arange(4096)//128 (contiguous groups), bias [4096], residual
[4,2048,4096].  out = x @ dequant(W) + bias + residual.

Sharding: data-parallel over tokens (M = B*S = 8192 rows -> 1024 rows/core);
every core keeps the full weight.  This keeps the x-transpose work (PE
transposes) and input DMA low; the per-core dequant of the full W overlaps
under the fp32r matmuls.

Layout trick: the contraction is processed in a PERMUTED k-order so the packed
int32 weight rows never need replication across partitions.  K splits into 4
super-tiles of 1024 (= 128 packed rows).  Within a super-tile, sub-matmul j
(j = 0..7) contracts k = 1024*T + 8*kp + j over partitions kp = 0..127:
  - rhs_j = ((wq_rows_T >> 4j) & 0xF) * scale  -- wq rows load 1:1 onto
    partitions (plain 2D DMA), one DVE shift/and + one DVE multiply per j
  - lhsT_j = xT slice; built in the prologue by PE-transposing x column slices
    x[:, j::8] so the permuted order falls out of the transpose for free
  - scale rows (group = 8T + kp//16) broadcast across partitions via one
    K=8 indicator matmul (E16^T @ scale_rows) into PSUM per (chunk, T)
  - GPTQ zero-points and bias fold into one rank-33 matmul: out -=
    Xg @ ((qz+1)*s); Xg (per-group sums of x) comes from DVE segmented
    reduces in the prologue, transposed on the PE
"""

import numpy as np

import concourse.bass as bass
import concourse.mybir as mybir
import concourse.tile as tile
from concourse import bacc
from concourse.alu_op_type import AluOpType
from concourse.bass_utils import run_bass_kernel_spmd
from concourse.masks import make_identity

F32 = mybir.dt.float32
F32R = mybir.dt.float16  # matmul dtype: fp16 streams 1cy/col w/ FWL; same 10-bit mantissa class as fp32r
I32 = mybir.dt.int32

B, S, K, N = 4, 2048, 4096, 4096
PACK = 8
GROUP = 128
G = K // GROUP          # 32 groups
NCORES = 8
M = (B * S) // NCORES   # 1024 rows per core
CHUNK = 512


def _build(M=M, K=K, N=N):
    G = K // GROUP
    MT = M // 128
    TS = K // 1024          # super-tiles of 1024 k
    NC_CH = N // CHUNK
    nc = bacc.Bacc(name="gptq_mm", dynamic_dma_scratch_size=2048)
    x_d = nc.declare_dram_parameter("x", [M, K], F32, isOutput=False)
    wq_d = nc.declare_dram_parameter("wq", [K // PACK, N], I32, isOutput=False)
    sc_d = nc.declare_dram_parameter("scales", [G, N], F32, isOutput=False)
    nzs_d = nc.declare_dram_parameter("nzs", [G + 1, N], F32, isOutput=False)
    e16_d = nc.declare_dram_parameter("e16", [8, 128], F32, isOutput=False)
    res_d = nc.declare_dram_parameter("resid", [M, N], F32, isOutput=False)
    out_d = nc.declare_dram_parameter("out", [M, N], F32, isOutput=True)

    with tile.TileContext(nc) as tc:
        with tc.tile_pool(name="const", bufs=1) as const:
            xt_all = const.tile([128, TS, 8, M], F32R, tag="xt")   # 128KB/part
            xgt = const.tile([G + 1, M], F32R, tag="xgt")
            e16_sb = const.tile([8, 128], F32R, tag="e16")

            # ---------------- prologue: transpose x, group sums ----------------
            with (
                tc.tile_pool(name="prol", bufs=3) as prol,
                tc.tile_pool(name="prolp", bufs=4, space="PSUM") as prolp,
                tc.tile_pool(name="prolp2", bufs=2, space="PSUM") as prolp2,
            ):
                ident = prol.tile([128, 128], F32, tag="ident", bufs=1)
                make_identity(nc, ident[:])
                e16_st = prol.tile([8, 128], F32, tag="e16st", bufs=1)
                nc.sync.dma_start(out=e16_st[:], in_=e16_d[:, :])
                nc.scalar.copy(e16_sb[:], e16_st[:])
                onesrow_f = prol.tile([1, M], F32, tag="onesrow", bufs=1)
                nc.vector.memset(onesrow_f[:], 1.0)
                nc.vector.tensor_copy(xgt[G:G + 1, :], onesrow_f[:])

                for mi in range(MT):
                    ms = slice(mi * 128, (mi + 1) * 128)
                    xg_sb = prol.tile([128, G], F32, tag="xgsb", bufs=2)
                    for t in range(TS):
                        xl = prol.tile([128, 1024], F32, tag="xl")
                        nc.sync.dma_start(
                            out=xl[:], in_=x_d[ms, t * 1024:(t + 1) * 1024]
                        )
                        xl3 = xl[:].rearrange("p (kp j) -> p kp j", j=8)
                        for j in range(8):
                            pt = prolp.tile([128, 128], F32, tag="pt")
                            nc.tensor.transpose(pt[:], xl3[:, :, j], ident[:])
                            dst = xt_all[:, t, j, ms]
                            if j % 2 == 0:
                                nc.vector.tensor_copy(dst, pt[:])
                            else:
                                nc.scalar.copy(dst, pt[:])
                        # per-group sums (groups 8t .. 8t+8)
                        nc.vector.tensor_reduce(
                            out=xg_sb[:, 8 * t:8 * t + 8],
                            in_=xl[:].rearrange("p (s v) -> p s v", v=GROUP),
                            axis=mybir.AxisListType.X,
                            op=AluOpType.add,
                        )
                    pxt = prolp2.tile([G, 128], F32, tag="pxt")
                    nc.tensor.transpose(pxt[:], xg_sb[:], ident[:])
                    nc.vector.tensor_copy(xgt[0:G, ms], pxt[:])

            # ---------------- steady state: chunks of CHUNK cols ----------------
            with (
                tc.tile_pool(name="wdq", bufs=40) as wdqp,
                tc.tile_pool(name="pk", bufs=4) as pkp,
                tc.tile_pool(name="u", bufs=4) as up,
                tc.tile_pool(name="stg", bufs=1) as stgp,
                tc.tile_pool(name="eout", bufs=4) as eoutp,
                tc.tile_pool(name="psums", bufs=2, space="PSUM") as psums,
                tc.tile_pool(name="psumm", bufs=6, space="PSUM") as psumm,
            ):
                for c in range(NC_CH):
                    cs = slice(c * CHUNK, (c + 1) * CHUNK)
                    # stage nzs for this chunk, cast to f32r
                    nzs_st = stgp.tile([G + 1, CHUNK], F32, tag="nzst", bufs=2)
                    nc.sync.dma_start(out=nzs_st[:], in_=nzs_d[:, cs])
                    nzs_r = stgp.tile([G + 1, CHUNK], F32R, tag="nzsr", bufs=2)
                    nc.scalar.copy(nzs_r[:], nzs_st[:])

                    wdq_tiles = []
                    for t in range(TS):
                        pk = pkp.tile([128, CHUNK], I32, tag="pk")
                        nc.sync.dma_start(
                            out=pk[:], in_=wq_d[128 * t:128 * (t + 1), cs]
                        )
                        # stage the 8 scale rows of this super-tile, cast to f32r
                        s8 = stgp.tile([8, CHUNK], F32, tag="s8", bufs=3)
                        nc.sync.dma_start(out=s8[:], in_=sc_d[8 * t:8 * t + 8, cs])
                        s8r = stgp.tile([8, CHUNK], F32R, tag="s8r", bufs=3)
                        nc.scalar.copy(s8r[:], s8[:])
                        # scale broadcast: ps_s[p, n] = scales[8t + p//16, n]
                        ps_s = psums.tile([128, CHUNK], F32, tag="pss")
                        nc.tensor.matmul(
                            ps_s[:], lhsT=e16_sb[:], rhs=s8r[:],
                            start=True, stop=True,
                        )
                        for j in range(8):
                            u = up.tile([128, CHUNK], I32, tag="u")
                            nc.vector.tensor_scalar(
                                out=u[:], in0=pk[:],
                                scalar1=4 * j, scalar2=0xF,
                                op0=AluOpType.logical_shift_right,
                                op1=AluOpType.bitwise_and,
                            )
                            wdq = wdqp.tile([128, CHUNK], F32R, tag="wdq")
                            nc.vector.tensor_tensor(
                                out=wdq[:], in0=u[:], in1=ps_s[:], op=AluOpType.mult,
                            )
                            wdq_tiles.append(wdq)

                    for mi in range(MT):
                        ms = slice(mi * 128, (mi + 1) * 128)
                        ps = psumm.tile([128, CHUNK], F32, tag="ps")
                        for t in range(TS):
                            for j in range(8):
                                nc.tensor.matmul(
                                    ps[:],
                                    lhsT=xt_all[:, t, j, ms],
                                    rhs=wdq_tiles[t * 8 + j][:],
                                    start=(t == 0 and j == 0), stop=False,
                                )
                        nc.tensor.matmul(
                            ps[:], lhsT=xgt[:, ms], rhs=nzs_r[:],
                            start=False, stop=True,
                        )
                        # epilogue at 256 granularity to keep tiles small
                        for h in range(CHUNK // 256):
                            hs = slice(h * 256, (h + 1) * 256)
                            hcs = slice(c * CHUNK + h * 256, c * CHUNK + (h + 1) * 256)
                            rt = eoutp.tile([128, 256], F32, tag="rt")
                            nc.sync.dma_start(out=rt[:], in_=res_d[ms, hcs])
                            ob = eoutp.tile([128, 256], F32, tag="ob")
                            nc.vector.tensor_tensor(
                                out=ob[:], in0=ps[:, hs], in1=rt[:], op=AluOpType.add,
                            )
                            nc.sync.dma_start(out=out_d[ms, hcs], in_=ob[:])

    nc.finalize()
    return nc


_NC_CACHE = None


def _get_nc():
    global _NC_CACHE
    if _NC_CACHE is None:
        _NC_CACHE = _build()
    return _NC_CACHE


def _host_prep(weight_scales, weight_zeros, bias):
    G_, N_ = weight_scales.shape
    jj = (np.arange(PACK, dtype=np.int32) * 4)
    qz = ((weight_zeros[:, :, None] >> jj[None, None, :]) & 0xF).reshape(G_, N_)
    nzs = np.concatenate(
        [-(qz + 1).astype(np.float32) * weight_scales, bias[None, :]], axis=0
    ).astype(np.float32)                                     # [G+1, N]
    # e16[r, p] = 1 if p//16 == r else 0
    e16 = np.repeat(np.eye(8, dtype=np.float32), 16, axis=1)  # [8, 128]
    return nzs, e16


def kernel(input, weight, weight_scales, weight_zeros, g_idx, bias, residual):
    input = np.asarray(input, dtype=np.float32)
    weight = np.ascontiguousarray(np.asarray(weight, dtype=np.int32))
    weight_scales = np.ascontiguousarray(np.asarray(weight_scales, dtype=np.float32))
    weight_zeros = np.asarray(weight_zeros, dtype=np.int32)
    g_idx = np.asarray(g_idx, dtype=np.int32)
    bias = np.asarray(bias, dtype=np.float32)
    residual = np.asarray(residual, dtype=np.float32)

    assert input.shape == (B, S, K) and weight.shape == (K // PACK, N)
    assert np.array_equal(g_idx, np.arange(K, dtype=np.int32) // GROUP), \
        "kernel assumes contiguous GPTQ groups (g_idx == arange(K)//group_size)"

    x = np.ascontiguousarray(input.reshape(B * S, K))
    resid = np.ascontiguousarray(residual.reshape(B * S, N))
    nzs, e16 = _host_prep(weight_scales, weight_zeros, bias)

    nc = _get_nc()
    in_maps = []
    for ci in range(NCORES):
        rs = slice(ci * M, (ci + 1) * M)
        in_maps.append(dict(
            x=np.ascontiguousarray(x[rs]),
            wq=weight,
            scales=weight_scales,
            nzs=nzs,
            e16=e16,
            resid=np.ascontiguousarray(resid[rs]),
        ))

    res = run_bass_kernel_spmd(nc, in_maps, core_ids=list(range(NCORES)))
    out = np.concatenate([r["out"] for r in res.results], axis=0)
    return out.reshape(B, S, N)


# revision 16
# speedup vs baseline: 1.4235x; 1.0126x over previous
"""GPTQ int4 dequant + matmul + bias + residual for Trainium2, 8 NeuronCores.

Problem (hardcoded): input [4,2048,4096] f32, qweight int32 [512,4096] (8 int4
along K per int32), scales [32,4096], qzeros int32 [32,512] (8 int4 along N),
g_idx = arange(4096)//128 (contiguous groups), bias [4096], residual
[4,2048,4096].  out = x @ dequant(W) + bias + residual.

Sharding: data-parallel over tokens (M = B*S = 8192 rows -> 1024 rows/core);
every core keeps the full weight.  This keeps the x-transpose work (PE
transposes) and input DMA low; the per-core dequant of the full W overlaps
under the fp32r matmuls.

Layout trick: the contraction is processed in a PERMUTED k-order so the packed
int32 weight rows never need replication across partitions.  K splits into 4
super-tiles of 1024 (= 128 packed rows).  Within a super-tile, sub-matmul j
(j = 0..7) contracts k = 1024*T + 8*kp + j over partitions kp = 0..127:
  - rhs_j = ((wq_rows_T >> 4j) & 0xF) * scale  -- wq rows load 1:1 onto
    partitions (plain 2D DMA), one DVE shift/and + one DVE multiply per j
  - lhsT_j = xT slice; built in the prologue by PE-transposing x column slices
    x[:, j::8] so the permuted order falls out of the transpose for free
  - scale rows (group = 8T + kp//16) broadcast across partitions via one
    K=8 indicator matmul (E16^T @ scale_rows) into PSUM per (chunk, T)
  - GPTQ zero-points and bias fold into one rank-33 matmul: out -=
    Xg @ ((qz+1)*s); Xg (per-group sums of x) comes from DVE segmented
    reduces in the prologue, transposed on the PE
"""

import numpy as np

import concourse.bass as bass
import concourse.mybir as mybir
import concourse.tile as tile
from concourse import bacc
from concourse.alu_op_type import AluOpType
from concourse.bass_utils import run_bass_kernel_spmd
from concourse.masks import make_identity

F32 = mybir.dt.float32
F32R = mybir.dt.float16  # matmul dtype: fp16 streams 1cy/col w/ FWL; same 10-bit mantissa class as fp32r
I32 = mybir.dt.int32

B, S, K, N = 4, 2048, 4096, 4096
PACK = 8
GROUP = 128
G = K // GROUP          # 32 groups
NCORES = 8
M = (B * S) // NCORES   # 1024 rows per core
CHUNK = 512


def _build(M=M, K=K, N=N):
    G = K // GROUP
    MT = M // 128
    TS = K // 1024          # super-tiles of 1024 k
    NC_CH = N // CHUNK
    nc = bacc.Bacc(name="gptq_mm", dynamic_dma_scratch_size=2048)
    x_d = nc.declare_dram_parameter("x", [M, K], F32, isOutput=False)
    wq_d = nc.declare_dram_parameter("wq", [K // PACK, N], I32, isOutput=False)
    sc_d = nc.declare_dram_parameter("scales", [G, N], F32, isOutput=False)
    nzs_d = nc.declare_dram_parameter("nzs", [G + 1, N], F32, isOutput=False)
    e16_d = nc.declare_dram_parameter("e16", [8, 128], F32, isOutput=False)
    res_d = nc.declare_dram_parameter("resid", [M, N], F32, isOutput=False)
    out_d = nc.declare_dram_parameter("out", [M, N], F32, isOutput=True)

    with tile.TileContext(nc) as tc:
        with tc.tile_pool(name="const", bufs=1) as const:
            xt_all = const.tile([128, TS, 8, M], F32R, tag="xt")   # 128KB/part
            xgt = const.tile([G + 1, M], F32R, tag="xgt")
            e16_sb = const.tile([8, 128], F32R, tag="e16")

            # ---------------- prologue: transpose x, group sums ----------------
            with (
                tc.tile_pool(name="prol", bufs=3) as prol,
                tc.tile_pool(name="prolp", bufs=4, space="PSUM") as prolp,
                tc.tile_pool(name="prolp2", bufs=2, space="PSUM") as prolp2,
            ):
                ident = prol.tile([128, 128], F32, tag="ident", bufs=1)
                make_identity(nc, ident[:])
                identh = prol.tile([128, 128], F32R, tag="identh", bufs=1)
                nc.vector.tensor_copy(identh[:], ident[:])
                e16_st = prol.tile([8, 128], F32, tag="e16st", bufs=1)
                nc.sync.dma_start(out=e16_st[:], in_=e16_d[:, :])
                nc.scalar.copy(e16_sb[:], e16_st[:])
                onesrow_f = prol.tile([1, M], F32, tag="onesrow", bufs=1)
                nc.vector.memset(onesrow_f[:], 1.0)
                nc.vector.tensor_copy(xgt[G:G + 1, :], onesrow_f[:])

                for mi in range(MT):
                    ms = slice(mi * 128, (mi + 1) * 128)
                    xg_sb = prol.tile([128, G], F32, tag="xgsb", bufs=2)
                    for t in range(TS):
                        xl = prol.tile([128, 1024], F32, tag="xl")
                        nc.sync.dma_start(
                            out=xl[:], in_=x_d[ms, t * 1024:(t + 1) * 1024]
                        )
                        xlh = prol.tile([128, 1024], F32R, tag="xlh", bufs=2)
                        nc.vector.tensor_copy(xlh[:], xl[:])
                        xl3 = xlh[:].rearrange("p (kp j) -> p kp j", j=8)
                        for j in range(8):
                            pt = prolp.tile([128, 128], F32R, tag="pt")
                            nc.tensor.transpose(pt[:], xl3[:, :, j], identh[:])
                            dst = xt_all[:, t, j, ms]
                            if j % 2 == 0:
                                nc.vector.tensor_copy(dst, pt[:])
                            else:
                                nc.scalar.copy(dst, pt[:])
                        # per-group sums (groups 8t .. 8t+8)
                        nc.vector.tensor_reduce(
                            out=xg_sb[:, 8 * t:8 * t + 8],
                            in_=xl[:].rearrange("p (s v) -> p s v", v=GROUP),
                            axis=mybir.AxisListType.X,
                            op=AluOpType.add,
                        )
                    pxt = prolp2.tile([G, 128], F32, tag="pxt")
                    nc.tensor.transpose(pxt[:], xg_sb[:], ident[:])
                    nc.vector.tensor_copy(xgt[0:G, ms], pxt[:])

            # ---------------- steady state: chunks of CHUNK cols ----------------
            with (
                tc.tile_pool(name="wdq", bufs=40) as wdqp,
                tc.tile_pool(name="pk", bufs=4) as pkp,
                tc.tile_pool(name="u", bufs=4) as up,
                tc.tile_pool(name="stg", bufs=1) as stgp,
                tc.tile_pool(name="eout", bufs=4) as eoutp,
                tc.tile_pool(name="psums", bufs=2, space="PSUM") as psums,
                tc.tile_pool(name="psumm", bufs=6, space="PSUM") as psumm,
            ):
                for c in range(NC_CH):
                    cs = slice(c * CHUNK, (c + 1) * CHUNK)
                    # stage nzs for this chunk, cast to f32r
                    nzs_st = stgp.tile([G + 1, CHUNK], F32, tag="nzst", bufs=2)
                    nc.sync.dma_start(out=nzs_st[:], in_=nzs_d[:, cs])
                    nzs_r = stgp.tile([G + 1, CHUNK], F32R, tag="nzsr", bufs=2)
                    nc.scalar.copy(nzs_r[:], nzs_st[:])

                    wdq_tiles = []
                    for t in range(TS):
                        pk = pkp.tile([128, CHUNK], I32, tag="pk")
                        nc.sync.dma_start(
                            out=pk[:], in_=wq_d[128 * t:128 * (t + 1), cs]
                        )
                        # stage the 8 scale rows of this super-tile, cast to f32r
                        s8 = stgp.tile([8, CHUNK], F32, tag="s8", bufs=3)
                        nc.sync.dma_start(out=s8[:], in_=sc_d[8 * t:8 * t + 8, cs])
                        s8r = stgp.tile([8, CHUNK], F32R, tag="s8r", bufs=3)
                        nc.scalar.copy(s8r[:], s8[:])
                        # scale broadcast: ps_s[p, n] = scales[8t + p//16, n]
                        ps_s = psums.tile([128, CHUNK], F32, tag="pss")
                        nc.tensor.matmul(
                            ps_s[:], lhsT=e16_sb[:], rhs=s8r[:],
                            start=True, stop=True,
                        )
                        for j in range(8):
                            u = up.tile([128, CHUNK], I32, tag="u")
                            nc.vector.tensor_scalar(
                                out=u[:], in0=pk[:],
                                scalar1=4 * j, scalar2=0xF,
                                op0=AluOpType.logical_shift_right,
                                op1=AluOpType.bitwise_and,
                            )
                            wdq = wdqp.tile([128, CHUNK], F32R, tag="wdq")
                            nc.vector.tensor_tensor(
                                out=wdq[:], in0=u[:], in1=ps_s[:], op=AluOpType.mult,
                            )
                            wdq_tiles.append(wdq)

                    for mi in range(MT):
                        ms = slice(mi * 128, (mi + 1) * 128)
                        ps = psumm.tile([128, CHUNK], F32, tag="ps")
                        for t in range(TS):
                            for j in range(8):
                                nc.tensor.matmul(
                                    ps[:],
                                    lhsT=xt_all[:, t, j, ms],
                                    rhs=wdq_tiles[t * 8 + j][:],
                                    start=(t == 0 and j == 0), stop=False,
                                )
                        nc.tensor.matmul(
                            ps[:], lhsT=xgt[:, ms], rhs=nzs_r[:],
                            start=False, stop=True,
                        )
                        # epilogue at 256 granularity to keep tiles small
                        for h in range(CHUNK // 256):
                            hs = slice(h * 256, (h + 1) * 256)
                            hcs = slice(c * CHUNK + h * 256, c * CHUNK + (h + 1) * 256)
                            rt = eoutp.tile([128, 256], F32, tag="rt")
                            nc.sync.dma_start(out=rt[:], in_=res_d[ms, hcs])
                            ob = eoutp.tile([128, 256], F32, tag="ob")
                            nc.vector.tensor_tensor(
                                out=ob[:], in0=ps[:, hs], in1=rt[:], op=AluOpType.add,
                            )
                            nc.sync.dma_start(out=out_d[ms, hcs], in_=ob[:])

    nc.finalize()
    return nc


_NC_CACHE = None


def _get_nc():
    global _NC_CACHE
    if _NC_CACHE is None:
        _NC_CACHE = _build()
    return _NC_CACHE


def _host_prep(weight_scales, weight_zeros, bias):
    G_, N_ = weight_scales.shape
    jj = (np.arange(PACK, dtype=np.int32) * 4)
    qz = ((weight_zeros[:, :, None] >> jj[None, None, :]) & 0xF).reshape(G_, N_)
    nzs = np.concatenate(
        [-(qz + 1).astype(np.float32) * weight_scales, bias[None, :]], axis=0
    ).astype(np.float32)                                     # [G+1, N]
    # e16[r, p] = 1 if p//16 == r else 0
    e16 = np.repeat(np.eye(8, dtype=np.float32), 16, axis=1)  # [8, 128]
    return nzs, e16


def kernel(input, weight, weight_scales, weight_zeros, g_idx, bias, residual):
    input = np.asarray(input, dtype=np.float32)
    weight = np.ascontiguousarray(np.asarray(weight, dtype=np.int32))
    weight_scales = np.ascontiguousarray(np.asarray(weight_scales, dtype=np.float32))
    weight_zeros = np.asarray(weight_zeros, dtype=np.int32)
    g_idx = np.asarray(g_idx, dtype=np.int32)
    bias = np.asarray(bias, dtype=np.float32)
    residual = np.asarray(residual, dtype=np.float32)

    assert input.shape == (B, S, K) and weight.shape == (K // PACK, N)
    assert np.array_equal(g_idx, np.arange(K, dtype=np.int32) // GROUP), \
        "kernel assumes contiguous GPTQ groups (g_idx == arange(K)//group_size)"

    x = np.ascontiguousarray(input.reshape(B * S, K))
    resid = np.ascontiguousarray(residual.reshape(B * S, N))
    nzs, e16 = _host_prep(weight_scales, weight_zeros, bias)

    nc = _get_nc()
    in_maps = []
    for ci in range(NCORES):
        rs = slice(ci * M, (ci + 1) * M)
        in_maps.append(dict(
            x=np.ascontiguousarray(x[rs]),
            wq=weight,
            scales=weight_scales,
            nzs=nzs,
            e16=e16,
            resid=np.ascontiguousarray(resid[rs]),
        ))

    res = run_bass_kernel_spmd(nc, in_maps, core_ids=list(range(NCORES)))
    out = np.concatenate([r["out"] for r in res.results], axis=0)
    return out.reshape(B, S, N)


# revision 17
# speedup vs baseline: 1.4393x; 1.0111x over previous
"""GPTQ int4 dequant + matmul + bias + residual for Trainium2, 8 NeuronCores.

Problem (hardcoded): input [4,2048,4096] f32, qweight int32 [512,4096] (8 int4
along K per int32), scales [32,4096], qzeros int32 [32,512] (8 int4 along N),
g_idx = arange(4096)//128 (contiguous groups), bias [4096], residual
[4,2048,4096].  out = x @ dequant(W) + bias + residual.

Sharding: data-parallel over tokens (M = B*S = 8192 rows -> 1024 rows/core);
every core keeps the full weight.  This keeps the x-transpose work (PE
transposes) and input DMA low; the per-core dequant of the full W overlaps
under the fp32r matmuls.

Layout trick: the contraction is processed in a PERMUTED k-order so the packed
int32 weight rows never need replication across partitions.  K splits into 4
super-tiles of 1024 (= 128 packed rows).  Within a super-tile, sub-matmul j
(j = 0..7) contracts k = 1024*T + 8*kp + j over partitions kp = 0..127:
  - rhs_j = ((wq_rows_T >> 4j) & 0xF) * scale  -- wq rows load 1:1 onto
    partitions (plain 2D DMA), one DVE shift/and + one DVE multiply per j
  - lhsT_j = xT slice; built in the prologue by PE-transposing x column slices
    x[:, j::8] so the permuted order falls out of the transpose for free
  - scale rows (group = 8T + kp//16) broadcast across partitions via one
    K=8 indicator matmul (E16^T @ scale_rows) into PSUM per (chunk, T)
  - GPTQ zero-points and bias fold into one rank-33 matmul: out -=
    Xg @ ((qz+1)*s); Xg (per-group sums of x) comes from DVE segmented
    reduces in the prologue, transposed on the PE
"""

import numpy as np

import concourse.bass as bass
import concourse.mybir as mybir
import concourse.tile as tile
from concourse import bacc
from concourse.alu_op_type import AluOpType
from concourse.bass_utils import run_bass_kernel_spmd
from concourse.masks import make_identity

F32 = mybir.dt.float32
F32R = mybir.dt.float16  # matmul dtype: fp16 streams 1cy/col w/ FWL; same 10-bit mantissa class as fp32r
I32 = mybir.dt.int32

B, S, K, N = 4, 2048, 4096, 4096
PACK = 8
GROUP = 128
G = K // GROUP          # 32 groups
NCORES = 8
M = (B * S) // NCORES   # 1024 rows per core
CHUNK = 512


def _build(M=M, K=K, N=N):
    G = K // GROUP
    MT = M // 128
    TS = K // 1024          # super-tiles of 1024 k
    NC_CH = N // CHUNK
    nc = bacc.Bacc(name="gptq_mm", dynamic_dma_scratch_size=2048)
    x_d = nc.declare_dram_parameter("x", [M, K], F32, isOutput=False)
    wq_d = nc.declare_dram_parameter("wq", [K // PACK, N], I32, isOutput=False)
    sc_d = nc.declare_dram_parameter("scales", [G, N], F32, isOutput=False)
    nzs_d = nc.declare_dram_parameter("nzs", [G + 1, N], F32, isOutput=False)
    e16_d = nc.declare_dram_parameter("e16", [8, 128], F32, isOutput=False)
    res_d = nc.declare_dram_parameter("resid", [M, N], F32, isOutput=False)
    out_d = nc.declare_dram_parameter("out", [M, N], F32, isOutput=True)

    with tile.TileContext(nc) as tc:
        with tc.tile_pool(name="const", bufs=1) as const:
            xt_all = const.tile([128, TS, 8, M], F32R, tag="xt")   # 128KB/part
            xgt = const.tile([G + 1, M], F32R, tag="xgt")
            e16_sb = const.tile([8, 128], F32R, tag="e16")

            # ---------------- prologue: transpose x, group sums ----------------
            with (
                tc.tile_pool(name="prol", bufs=4) as prol,
                tc.tile_pool(name="prolp", bufs=5, space="PSUM") as prolp,
                tc.tile_pool(name="prolp2", bufs=2, space="PSUM") as prolp2,
            ):
                ident = prol.tile([128, 128], F32, tag="ident", bufs=1)
                make_identity(nc, ident[:])
                identh = prol.tile([128, 128], F32R, tag="identh", bufs=1)
                nc.vector.tensor_copy(identh[:], ident[:])
                e16_st = prol.tile([8, 128], F32, tag="e16st", bufs=1)
                nc.sync.dma_start(out=e16_st[:], in_=e16_d[:, :])
                nc.scalar.copy(e16_sb[:], e16_st[:])
                onesrow_f = prol.tile([1, M], F32, tag="onesrow", bufs=1)
                nc.vector.memset(onesrow_f[:], 1.0)
                nc.vector.tensor_copy(xgt[G:G + 1, :], onesrow_f[:])

                for mi in range(MT):
                    ms = slice(mi * 128, (mi + 1) * 128)
                    xg_sb = prol.tile([128, G], F32, tag="xgsb", bufs=2)
                    for t in range(TS):
                        xl = prol.tile([128, 1024], F32, tag="xl")
                        nc.sync.dma_start(
                            out=xl[:], in_=x_d[ms, t * 1024:(t + 1) * 1024]
                        )
                        xlh = prol.tile([128, 1024], F32R, tag="xlh", bufs=3)
                        nc.vector.tensor_copy(xlh[:], xl[:])
                        xl3 = xlh[:].rearrange("p (kp j) -> p kp j", j=8)
                        for j in range(8):
                            pt = prolp.tile([128, 128], F32R, tag="pt")
                            nc.tensor.transpose(pt[:], xl3[:, :, j], identh[:])
                            dst = xt_all[:, t, j, ms]
                            if j % 2 == 0:
                                nc.vector.tensor_copy(dst, pt[:])
                            else:
                                nc.scalar.copy(dst, pt[:])
                        # per-group sums (groups 8t .. 8t+8)
                        nc.vector.tensor_reduce(
                            out=xg_sb[:, 8 * t:8 * t + 8],
                            in_=xl[:].rearrange("p (s v) -> p s v", v=GROUP),
                            axis=mybir.AxisListType.X,
                            op=AluOpType.add,
                        )
                    pxt = prolp2.tile([G, 128], F32, tag="pxt")
                    nc.tensor.transpose(pxt[:], xg_sb[:], ident[:])
                    nc.vector.tensor_copy(xgt[0:G, ms], pxt[:])

            # ---------------- steady state: chunks of CHUNK cols ----------------
            with (
                tc.tile_pool(name="wdq", bufs=48) as wdqp,
                tc.tile_pool(name="pk", bufs=4) as pkp,
                tc.tile_pool(name="u", bufs=4) as up,
                tc.tile_pool(name="stg", bufs=1) as stgp,
                tc.tile_pool(name="eout", bufs=6) as eoutp,
                tc.tile_pool(name="psums", bufs=2, space="PSUM") as psums,
                tc.tile_pool(name="psumm", bufs=6, space="PSUM") as psumm,
            ):
                for c in range(NC_CH):
                    cs = slice(c * CHUNK, (c + 1) * CHUNK)
                    # stage nzs for this chunk, cast to f32r
                    nzs_st = stgp.tile([G + 1, CHUNK], F32, tag="nzst", bufs=2)
                    nc.sync.dma_start(out=nzs_st[:], in_=nzs_d[:, cs])
                    nzs_r = stgp.tile([G + 1, CHUNK], F32R, tag="nzsr", bufs=2)
                    nc.scalar.copy(nzs_r[:], nzs_st[:])

                    wdq_tiles = []
                    for t in range(TS):
                        pk = pkp.tile([128, CHUNK], I32, tag="pk")
                        nc.sync.dma_start(
                            out=pk[:], in_=wq_d[128 * t:128 * (t + 1), cs]
                        )
                        # stage the 8 scale rows of this super-tile, cast to f32r
                        s8 = stgp.tile([8, CHUNK], F32, tag="s8", bufs=3)
                        nc.sync.dma_start(out=s8[:], in_=sc_d[8 * t:8 * t + 8, cs])
                        s8r = stgp.tile([8, CHUNK], F32R, tag="s8r", bufs=3)
                        nc.scalar.copy(s8r[:], s8[:])
                        # scale broadcast: ps_s[p, n] = scales[8t + p//16, n]
                        ps_s = psums.tile([128, CHUNK], F32, tag="pss")
                        nc.tensor.matmul(
                            ps_s[:], lhsT=e16_sb[:], rhs=s8r[:],
                            start=True, stop=True,
                        )
                        for j in range(8):
                            u = up.tile([128, CHUNK], I32, tag="u")
                            nc.vector.tensor_scalar(
                                out=u[:], in0=pk[:],
                                scalar1=4 * j, scalar2=0xF,
                                op0=AluOpType.logical_shift_right,
                                op1=AluOpType.bitwise_and,
                            )
                            wdq = wdqp.tile([128, CHUNK], F32R, tag="wdq")
                            nc.vector.tensor_tensor(
                                out=wdq[:], in0=u[:], in1=ps_s[:], op=AluOpType.mult,
                            )
                            wdq_tiles.append(wdq)

                    for mi in range(MT):
                        ms = slice(mi * 128, (mi + 1) * 128)
                        ps = psumm.tile([128, CHUNK], F32, tag="ps")
                        for t in range(TS):
                            for j in range(8):
                                nc.tensor.matmul(
                                    ps[:],
                                    lhsT=xt_all[:, t, j, ms],
                                    rhs=wdq_tiles[t * 8 + j][:],
                                    start=(t == 0 and j == 0), stop=False,
                                )
                        nc.tensor.matmul(
                            ps[:], lhsT=xgt[:, ms], rhs=nzs_r[:],
                            start=False, stop=True,
                        )
                        # epilogue at 256 granularity to keep tiles small
                        for h in range(CHUNK // 256):
                            hs = slice(h * 256, (h + 1) * 256)
                            hcs = slice(c * CHUNK + h * 256, c * CHUNK + (h + 1) * 256)
                            rt = eoutp.tile([128, 256], F32, tag="rt")
                            nc.sync.dma_start(out=rt[:], in_=res_d[ms, hcs])
                            ob = eoutp.tile([128, 256], F32, tag="ob")
                            nc.vector.tensor_tensor(
                                out=ob[:], in0=ps[:, hs], in1=rt[:], op=AluOpType.add,
                            )
                            nc.sync.dma_start(out=out_d[ms, hcs], in_=ob[:])

    nc.finalize()
    return nc


_NC_CACHE = None


def _get_nc():
    global _NC_CACHE
    if _NC_CACHE is None:
        _NC_CACHE = _build()
    return _NC_CACHE


def _host_prep(weight_scales, weight_zeros, bias):
    G_, N_ = weight_scales.shape
    jj = (np.arange(PACK, dtype=np.int32) * 4)
    qz = ((weight_zeros[:, :, None] >> jj[None, None, :]) & 0xF).reshape(G_, N_)
    nzs = np.concatenate(
        [-(qz + 1).astype(np.float32) * weight_scales, bias[None, :]], axis=0
    ).astype(np.float32)                                     # [G+1, N]
    # e16[r, p] = 1 if p//16 == r else 0
    e16 = np.repeat(np.eye(8, dtype=np.float32), 16, axis=1)  # [8, 128]
    return nzs, e16


def kernel(input, weight, weight_scales, weight_zeros, g_idx, bias, residual):
    input = np.asarray(input, dtype=np.float32)
    weight = np.ascontiguousarray(np.asarray(weight, dtype=np.int32))
    weight_scales = np.ascontiguousarray(np.asarray(weight_scales, dtype=np.float32))
    weight_zeros = np.asarray(weight_zeros, dtype=np.int32)
    g_idx = np.asarray(g_idx, dtype=np.int32)
    bias = np.asarray(bias, dtype=np.float32)
    residual = np.asarray(residual, dtype=np.float32)

    assert input.shape == (B, S, K) and weight.shape == (K // PACK, N)
    assert np.array_equal(g_idx, np.arange(K, dtype=np.int32) // GROUP), \
        "kernel assumes contiguous GPTQ groups (g_idx == arange(K)//group_size)"

    x = np.ascontiguousarray(input.reshape(B * S, K))
    resid = np.ascontiguousarray(residual.reshape(B * S, N))
    nzs, e16 = _host_prep(weight_scales, weight_zeros, bias)

    nc = _get_nc()
    in_maps = []
    for ci in range(NCORES):
        rs = slice(ci * M, (ci + 1) * M)
        in_maps.append(dict(
            x=np.ascontiguousarray(x[rs]),
            wq=weight,
            scales=weight_scales,
            nzs=nzs,
            e16=e16,
            resid=np.ascontiguousarray(resid[rs]),
        ))

    res = run_bass_kernel_spmd(nc, in_maps, core_ids=list(range(NCORES)))
    out = np.concatenate([r["out"] for r in res.results], axis=0)
    return out.reshape(B, S, N)
